# revision 2
# baseline (speedup 1.0000x reference)
"""Trainium2 Bass kernel for nn_KeyNet (gnn_message_passing).

Strategy (per spec sharding hint): shard the 125 anchors across 8 NeuronCores
(16 anchors/core, last core padded). Each core runs the per-point MLP chain for
its anchors and the softmax-weighted pooling, in a hybrid layout:

  - conv1 is folded into a per-anchor bias:  x1 = relu(x@W1.T + (b1 - anc@W1.T))
    so the shared x@W1.T matmul is computed once per core and each anchor's x1
    is a single ScalarE activation with a per-partition bias.
  - e-chain (e1, e2) is anchor-independent: computed once per core.
  - conv2/conv5 run channels-on-partitions (weights stationary).
  - all_conv1 runs transposed (activations as stationary lhsT, W.T as moving
    rhs) so its output z^T lands points-on-partitions.  The softmax weight
    e_p = exp(-|x_p - anc|) is then a per-partition ScalarE scale fused into
    the LeakyRelu eviction (lrelu(e*z) = e*lrelu(z) since e > 0), and the
    pooling sum over points is a ones-vector matmul contraction on the PE.
  - all_conv2 and the tiny kp/att heads are applied after pooling (all_conv2 is
    linear, so it commutes with the weighted sum) on the host, together with
    the anchor-softmax, argmin and final gather (~0.02% of the FLOPs).
  - all heavy matmuls use float32r (full PE rate, ~1e-4 relative rounding).

Device outputs per core: pooled[16, 320] = sum_p e_p * lrelu(all_conv1(allf)_p)
and sumexp[1, 16] = sum_p e_p.  Host divides, applies heads, softmax, argmin.
"""

import os
import numpy as np

import concourse.bass as bass
import concourse.tile as tile
from concourse import bacc, mybir
from concourse.bass_utils import run_bass_kernel_spmd

F32 = mybir.dt.float32
F32R = mybir.dt.float32r
AF = mybir.ActivationFunctionType
ALU = mybir.AluOpType

A, P, K, DI, HS = 125, 1024, 8, 32, 96
N_CORES = 8
A_CORE = 16  # anchors per core (8*16 = 128 >= 125, padded)
NEG_SLOPE = 0.01
D2_EPS = 1e-4  # guards sqrt against tiny negative d2 from f32r rounding

_cached = {}

# dram tensor specs: name -> (shape, dtype)
_INPUT_SPECS = {
    "xT": ([3, P], F32R),
    "m2xT": ([3, P], F32R),
    "xsq_ones": ([2, P], F32R),
    "embT": ([DI, P], F32R),
    "ones128": ([128, 1], F32R),
    "W1T": ([3, 64], F32R),
    "We1T": ([32, 64], F32R),
    "be1": ([64, 1], F32),
    "We2T": ([64, 128], F32R),
    "be2": ([128, 1], F32),
    "W2T": ([64, 128], F32R),
    "b2": ([128, 1], F32),
    "W5T0": ([128, 256], F32R),
    "W5T1": ([128, 256], F32R),
    "b5a": ([128, 1], F32),
    "b5b": ([128, 1], F32),
    "Ra": ([65, 320], F32R),
    "Rb": ([64, 320], F32R),
    "Rc": ([128, 320], F32R),
    "Rd": ([128, 320], F32R),
    "Re": ([128, 320], F32R),
    "Rf": ([128, 320], F32R),
    "ancT16": ([3, A_CORE], F32R),
    "rhs2x16": ([2, A_CORE], F32R),
    "bias1T": ([65, A_CORE], F32),
}


def _build_program():
    nc = bacc.Bacc("TRN2", target_bir_lowering=False, debug=False,
                   num_devices=N_CORES)
    d = {}
    for name, (shape, dt) in _INPUT_SPECS.items():
        d[name] = nc.dram_tensor(name, shape, dt, kind="ExternalInput").ap()
    pooled_d = nc.dram_tensor("pooled", [A_CORE, 320], F32,
                              kind="ExternalOutput").ap()
    sumexp_d = nc.dram_tensor("sumexp", [1, A_CORE], F32,
                              kind="ExternalOutput").ap()

    with tile.TileContext(nc) as tc:
        with (
            tc.tile_pool(name="consts", bufs=1) as cp,
            tc.tile_pool(name="work", bufs=2) as wp,
            tc.tile_pool(name="gw", bufs=3) as gp,
            tc.tile_pool(name="rows", bufs=3) as rp,
            tc.tile_pool(name="pc", bufs=3, space="PSUM") as pcp,
            tc.tile_pool(name="pz", bufs=2, space="PSUM") as pzp,
            tc.tile_pool(name="pp", bufs=2, space="PSUM") as ppp,
        ):
            sb = {}
            for name, (shape, dt) in _INPUT_SPECS.items():
                t = cp.tile(shape, dt, tag=name)
                nc.sync.dma_start(t[:], d[name][:])
                sb[name] = t

            def mm(out, lhsT, rhs, start, stop):
                nc.tensor.matmul(out[:], lhsT, rhs, start=start, stop=stop,
                                 skip_group_check=True)

            # ---- shared phase (once per core) ----
            # e1 = relu(We1 @ embT + be1)   [64, 1024]
            e1 = cp.tile([64, P], F32R, tag="e1")
            for n in range(2):
                ns = bass.ts(n, 512)
                ps = pcp.tile([64, 512], F32, tag="pc")
                mm(ps, sb["We1T"][:], sb["embT"][:, ns], True, True)
                nc.scalar.activation(e1[:, ns], ps[:], AF.Relu,
                                     bias=sb["be1"][:, 0:1])
            # e2 = relu(We2 @ e1 + be2)     [128, 1024]
            e2 = cp.tile([128, P], F32R, tag="e2")
            for n in range(2):
                ns = bass.ts(n, 512)
                ps = pcp.tile([128, 512], F32, tag="pc")
                mm(ps, sb["We2T"][:], e1[:, ns], True, True)
                nc.scalar.activation(e2[:, ns], ps[:], AF.Relu,
                                     bias=sb["be2"][:, 0:1])
            # xw1_aug rows 0:64 = W1 @ x^T; row 64 = 1.0
            xw1 = cp.tile([65, P], F32, tag="xw1")
            for n in range(2):
                ns = bass.ts(n, 512)
                ps = pcp.tile([64, 512], F32, tag="pc")
                mm(ps, sb["W1T"][:], sb["xT"][:, ns], True, True)
                nc.scalar.copy(xw1[0:64, ns], ps[:])
            nc.vector.memset(xw1[64:65, :], 1.0)

            # normT[p, a] = sqrt(|x_p - anc_a|^2 + eps), chunked over points
            normT = cp.tile([128, 8 * A_CORE], F32, tag="normT")
            for j in range(8):
                js = bass.ts(j, 128)
                ps = pzp.tile([128, A_CORE], F32, tag="pz")
                mm(ps, sb["m2xT"][:, js], sb["ancT16"][:], True, False)
                mm(ps, sb["xsq_ones"][:, js], sb["rhs2x16"][:], False, True)
                nc.scalar.activation(normT[:, bass.ts(j, A_CORE)], ps[:],
                                     AF.Sqrt)
            # eT = exp(-normT); f32r copy for matmul use
            eT = cp.tile([128, 8 * A_CORE], F32, tag="eT")
            nc.scalar.activation(eT[:], normT[:], AF.Exp, scale=-1.0)
            eTr = cp.tile([128, 8 * A_CORE], F32R, tag="eTr")
            nc.vector.tensor_copy(eTr[:], eT[:])
            # sumexp[1, 16] = sum_p e
            pse = ppp.tile([1, A_CORE], F32, tag="pp")
            for j in range(8):
                mm(pse, sb["ones128"][:], eTr[:, bass.ts(j, A_CORE)],
                   j == 0, j == 7)
            se_row = rp.tile([1, A_CORE], F32, tag="serow")
            nc.vector.tensor_copy(se_row[:], pse[0:1, :])
            nc.sync.dma_start(sumexp_d[:], se_row[:])

            # ---- per-anchor phase ----
            for a in range(A_CORE):
                # x1_aug = relu(xw1_aug + bias_a)  [65, 1024] (row 64 == 1)
                x1 = wp.tile([65, P], F32R, tag="x1")
                nc.scalar.activation(x1[:], xw1[:], AF.Relu,
                                     bias=sb["bias1T"][:, a:a + 1])
                # x2 = relu(W2 @ x1 + b2)  [128, 1024]
                x2 = wp.tile([128, P], F32R, tag="x2")
                for n in range(2):
                    ns = bass.ts(n, 512)
                    ps = pcp.tile([128, 512], F32, tag="pc")
                    mm(ps, sb["W2T"][:], x1[0:64, ns], True, True)
                    nc.vector.tensor_scalar(x2[:, ns], ps[:],
                                            sb["b2"][:, 0:1], 0.0,
                                            ALU.add, ALU.max)
                # x5 = relu(W5 @ [x2; e2] + b5)  [256, 1024] as two m-tiles
                x5 = [wp.tile([128, P], F32R, tag="x5a", name=f"x5a_{a}"),
                      wp.tile([128, P], F32R, tag="x5b", name=f"x5b_{a}")]
                b5s = [sb["b5a"], sb["b5b"]]
                for m in range(2):
                    ms = bass.ts(m, 128)
                    for n in range(2):
                        ns = bass.ts(n, 512)
                        ps = pcp.tile([128, 512], F32, tag="pc")
                        mm(ps, sb["W5T0"][:, ms], x2[:, ns], True, False)
                        mm(ps, sb["W5T1"][:, ms], e2[:, ns], False, True)
                        nc.vector.tensor_scalar(x5[m][:, ns], ps[:],
                                                b5s[m][:, 0:1], 0.0,
                                                ALU.add, ALU.max)
                # z^T chunks + fused e*lrelu eviction + pooling matmul
                ppool = ppp.tile([1, 320], F32, tag="pp")
                for j in range(8):
                    js = bass.ts(j, 128)
                    pz = pzp.tile([128, 320], F32, tag="pz")
                    mm(pz, x1[:, js], sb["Ra"][:], True, False)
                    mm(pz, e1[:, js], sb["Rb"][:], False, False)
                    mm(pz, x2[:, js], sb["Rc"][:], False, False)
                    mm(pz, e2[:, js], sb["Rd"][:], False, False)
                    mm(pz, x5[0][:, js], sb["Re"][:], False, False)
                    mm(pz, x5[1][:, js], sb["Rf"][:], False, True)
                    g = gp.tile([128, 320], F32R, tag="g")
                    nc.scalar.activation(
                        g[:], pz[:], AF.Lrelu,
                        scale=eT[:, j * A_CORE + a: j * A_CORE + a + 1],
                        alpha=NEG_SLOPE)
                    mm(ppool, sb["ones128"][:], g[:], j == 0, j == 7)
                prow = rp.tile([1, 320], F32, tag="prow")
                nc.vector.tensor_copy(prow[:], ppool[0:1, :])
                nc.sync.dma_start(pooled_d[a:a + 1, :], prow[:])

    nc.compile()
    return nc


def _host_inputs(out_img, choose, x, anchor, scale, gt_t, params):
    """Build the per-core in_maps (all host math in float64 -> float32)."""
    f32 = np.float32
    out_img = np.asarray(out_img, f32)
    choose = np.asarray(choose).reshape(-1).astype(np.int64)
    x0 = np.asarray(x, f32)[0]                    # [P, 3]
    anc = np.asarray(anchor, f32)[0]              # [A, 3]
    W = {k: (np.asarray(v[0], f32), np.asarray(v[1], f32))
         for k, v in params.items()}

    W1, b1 = W["conv1"]; We1, be1 = W["e_conv1"]
    W2, b2 = W["conv2"]; We2, be2 = W["e_conv2"]
    W5, b5 = W["conv5"]; Wa1, ba1 = W["all_conv1"]

    embT = out_img.reshape(DI, HS * HS)[:, choose]          # [32, P]
    xT = x0.T.copy()                                        # [3, P]
    xsq = (x0 * x0).sum(1)                                  # [P]
    Wa1T = Wa1.T.copy()                                     # [640, 320]
    W5T = W5.T.copy()                                       # [256, 256]

    shared = {
        "xT": xT,
        "m2xT": (-2.0 * xT),
        "xsq_ones": np.stack([xsq, np.ones(P, f32)]),
        "embT": embT,
        "ones128": np.ones((128, 1), f32),
        "W1T": W1.T.copy(),
        "We1T": We1.T.copy(),
        "be1": be1[:, None],
        "We2T": We2.T.copy(),
        "be2": be2[:, None],
        "W2T": W2.T.copy(),
        "b2": b2[:, None],
        "W5T0": W5T[0:128], "W5T1": W5T[128:256],
        "b5a": b5[0:128, None], "b5b": b5[128:256, None],
        "Ra": np.vstack([Wa1T[0:64], ba1[None, :]]),
        "Rb": Wa1T[64:128],
        "Rc": Wa1T[128:256],
        "Rd": Wa1T[256:384],
        "Re": Wa1T[384:512],
        "Rf": Wa1T[512:640],
    }
    shared = {k: np.ascontiguousarray(v, f32) for k, v in shared.items()}

    in_maps = []
    for c in range(N_CORES):
        ids = np.clip(np.arange(c * A_CORE, (c + 1) * A_CORE), 0, A - 1)
        anc_c = anc[ids]                                    # [16, 3]
        ancsq = (anc_c * anc_c).sum(1) + D2_EPS
        bias1 = b1[None, :] - anc_c @ W1.T                  # [16, 64]
        m = dict(shared)
        m["ancT16"] = np.ascontiguousarray(anc_c.T, f32)
        m["rhs2x16"] = np.stack([np.ones(A_CORE, f32),
                                 ancsq.astype(f32)])
        m["bias1T"] = np.vstack([bias1.T,
                                 np.zeros((1, A_CORE), f32)]).astype(f32)
        in_maps.append(m)
    return in_maps


def _lrelu(v):
    return np.where(v >= 0, v, NEG_SLOPE * v)


def kernel(out_img, choose, x, anchor, scale, gt_t, params):
    if "nc" not in _cached:
        _cached["nc"] = _build_program()
    nc = _cached["nc"]

    in_maps = _host_inputs(out_img, choose, x, anchor, scale, gt_t, params)
    trace = bool(os.environ.get("KEYNET_TRACE"))
    res = run_bass_kernel_spmd(nc, in_maps, core_ids=list(range(N_CORES)),
                               trace=trace)
    _cached["last_exec_time_ns"] = res.exec_time_ns

    pooled = np.concatenate([res.results[c]["pooled"] for c in range(N_CORES)],
                            axis=0)[:A].astype(np.float64)       # [125, 320]
    sumexp = np.concatenate([res.results[c]["sumexp"][0]
                             for c in range(N_CORES)])[:A]       # [125]

    # host: post-pool heads (float64)
    f64 = np.float64
    anc = np.asarray(anchor, np.float32)[0].astype(f64)          # [125, 3]
    W = {k: (np.asarray(v[0], np.float32).astype(f64),
             np.asarray(v[1], np.float32).astype(f64))
         for k, v in params.items()}
    Wa2, ba2 = W["all_conv2"]
    Wk1, bk1 = W["kp_1"]; Wk2, bk2 = W["kp_2"]
    Wt1, bt1 = W["att_1"]; Wt2, bt2 = W["att_2"]

    feat = (pooled / sumexp[:, None]) @ Wa2.T + ba2              # [125, 160]
    kp = _lrelu(feat @ Wk1.T + bk1) @ Wk2.T + bk2                # [125, 24]
    kp = kp.reshape(A, K, 3) + anc[:, None, :]
    att = (_lrelu(feat @ Wt1.T + bt1) @ Wt2.T + bt2).reshape(A)  # [125]
    att = att - att.max()
    att_x = np.exp(att); att_x /= att_x.sum()

    s = np.asarray(scale, np.float32).reshape(1, 3).astype(f64)
    output_anchor = anc * s
    gt = np.asarray(gt_t, np.float32).reshape(1, 3).astype(f64)
    min_choose = int(np.argmin(np.linalg.norm(output_anchor - gt, axis=1)))
    all_kp_x = kp[min_choose] * s                                # [8, 3]

    return (all_kp_x[None].astype(np.float32),
            output_anchor[None].astype(np.float32),
            att_x[None].astype(np.float32))
